# revision 1
# baseline (speedup 1.0000x reference)
"""DGDAGRNN (gated DAG RNN) Trainium2 Bass kernel, 8-core SPMD.

Structure: 2 rounds x (forward, backward) topological sweeps over 8 layers
(32 sequential layer-steps). Nodes are sharded 8 ways within each layer
(512 nodes per core per layer). Per step and per core:

  - gather message rows for this core's destination edges from a Shared
    DRAM message table (bf16, 128-float padded rows) via transpose-mode
    dma_gather -> feature-major [128, E_pad]
  - block-padded segment-sum on the vector engine -> agg^T [128, 512]
  - GRU cell entirely feature-major; gi+gh fused via PSUM accumulation;
    round-1 forward projector folded into the input weights (Wih_f @ Wp)
  - messages for the just-updated nodes computed node-major (stationary
    h^T trick, gate bias folded in via a ones-row), AllGathered into the
    next slab of the message table

All graph-dependent sizes (padded edge counts, per-block reduce widths)
are baked into the program at build time from the actual edge_index.
"""
import sys
import os

sys.path.insert(0, "/opt/trn_rl_repo")

import numpy as np
import ml_dtypes

N, E, L, VHS, NVT = 32768, 131072, 8, 100, 3
PER = N // L            # 4096 nodes per layer
C = 8                   # cores
SH = PER // C           # 512 nodes per core per layer
DBLK = 64               # dst nodes per reduce block
NBLK = SH // DBLK
NROUNDS = 2
TROWS = 1 + (L - 1) * PER
bf16 = ml_dtypes.bfloat16

_CACHE = {}


def _wrap_idx(idx):
    """int idx[n] (n % 16 == 0) -> [128, n/16] int16 wrapped in 16
    partitions, replicated across the 8 gpsimd cores."""
    w = idx.reshape(-1, 16).T.astype(np.int16)
    return np.tile(w, (8, 1))


def _build_tables(src, dst, nl):
    """Gather index tables. Block widths Ks are shared across cores
    (max over cores) so one SPMD program fits all cores.

    Returns steps[(sweep, l)] = dict(Ks, epad, idx=[per-core int32 arrays]).
    """
    steps = {}
    nl_src = nl[src]
    nl_dst = nl[dst]
    for sweep in ("f", "b"):
        if sweep == "f":
            agg_node, row_of, step_of = dst, src + 1, nl_dst
        else:
            agg_node, row_of, step_of = src, dst - PER + 1, (L - 1) - nl_src
        for l in range(1, L):
            sel = np.nonzero(step_of == l)[0]
            a = agg_node[sel]
            r = row_of[sel]
            w = a % PER
            core = w // SH
            dloc = w % SH
            buckets = [[[] for _ in range(SH)] for _ in range(C)]
            for c, dl, rr in zip(core, dloc, r):
                buckets[c][dl].append(int(rr))
            Ks = []
            for b in range(NBLK):
                K = 1
                for c in range(C):
                    for d in range(b * DBLK, (b + 1) * DBLK):
                        K = max(K, len(buckets[c][d]))
                Ks.append(K)
            used = DBLK * sum(Ks)
            epad = -(-used // 128) * 128
            idxs = []
            for c in range(C):
                arr = np.zeros(epad, np.int32)
                off = 0
                for b, K in enumerate(Ks):
                    for d in range(b * DBLK, (b + 1) * DBLK):
                        ebs = buckets[c][d]
                        arr[off:off + len(ebs)] = ebs
                        off += K
                idxs.append(arr)
            steps[(sweep, l)] = dict(Ks=Ks, epad=epad, idx=idxs)
    return steps


BIAS_COLS = ["br_f", "bz_f", "bn_i_f", "bn_h_f", "br_f1", "bz_f1",
             "bn_i_f1", "br_b", "bz_b", "bn_i_b", "bn_h_b"]


def _prep(inp):
    src = inp["edge_index"][0].astype(np.int64)
    dst = inp["edge_index"][1].astype(np.int64)
    nl = inp["node_layer"].astype(np.int64)
    assert np.array_equal(nl, np.arange(N) // PER), "unexpected node_layer"
    assert dst.min() >= PER and (nl[src] < nl[dst]).all(), "not a layer DAG"

    steps = _build_tables(src, dst, nl)
    # concatenate per-core gather indices; record slot offsets
    order = [(s, l) for s in ("f", "b") for l in range(1, L)]
    off = 0
    for key in order:
        steps[key]["off"] = off
        off += steps[key]["epad"] // 16
    tot_slots = off
    gidx = []
    for c in range(C):
        parts = [_wrap_idx(steps[key]["idx"][c]) for key in order]
        gidx.append(np.concatenate(parts, axis=1))

    W = {}
    f32 = np.float32
    for d in ("f", "b"):
        W[f"WhhT_{d}"] = inp[f"Whh_{d}"].T.astype(f32)
        W[f"WihT_{d}"] = inp[f"Wih_{d}"].T.astype(f32)
        Wg, bg, Wm = inp[f"Wg_{d}"], inp[f"bg_{d}"], inp[f"Wm_{d}"]
        W[f"WgmT_{d}"] = np.concatenate([
            np.concatenate([Wg.T, bg[None, :]], 0),
            np.concatenate([Wm.T, np.zeros((1, VHS))], 0)], 1).astype(f32)
    M = (inp["Wih_f"] @ inp["Wp"]).astype(f32)
    b2 = (inp["Wih_f"] @ inp["bp"]).astype(f32)
    W["WihT_f1"] = M.T.copy()
    bias = {}
    for d in ("f", "b"):
        bih, bhh = inp[f"bih_{d}"], inp[f"bhh_{d}"]
        bias[f"br_{d}"] = bih[:VHS] + bhh[:VHS]
        bias[f"bz_{d}"] = bih[VHS:2 * VHS] + bhh[VHS:2 * VHS]
        bias[f"bn_i_{d}"] = bih[2 * VHS:]
        bias[f"bn_h_{d}"] = bhh[2 * VHS:]
    bias["br_f1"] = bias["br_f"] + b2[:VHS]
    bias["bz_f1"] = bias["bz_f"] + b2[VHS:2 * VHS]
    bias["bn_i_f1"] = bias["bn_i_f"] + b2[2 * VHS:]
    W["bias_pack"] = np.stack(
        [bias[k] for k in BIAS_COLS], axis=1).astype(f32)   # [100, 11]

    x = inp["x"].astype(f32)
    xT = np.zeros((C, NVT, L * SH), f32)
    for c in range(C):
        for l in range(L):
            rows = l * PER + c * SH + np.arange(SH)
            xT[c][:, l * SH:(l + 1) * SH] = x[rows].T

    in_maps = []
    for c in range(C):
        m = {"xT": xT[c], "gidx": gidx[c]}
        for k, v in W.items():
            m[k] = v
        in_maps.append(m)
    meta = dict(steps=steps, tot_slots=tot_slots,
                wshapes={k: list(v.shape) for k, v in W.items()})
    return in_maps, meta


def _build_program(meta):
    import concourse.bacc as bacc
    import concourse.mybir as mybir
    import concourse.tile as tile

    dt = mybir.dt
    ACT = mybir.ActivationFunctionType
    ALU = mybir.AluOpType
    steps = meta["steps"]

    nc = bacc.Bacc("TRN2", target_bir_lowering=False, debug=False,
                   num_devices=C)
    xT_d = nc.dram_tensor("xT", [NVT, L * SH], dt.float32,
                          kind="ExternalInput")
    gidx_d = nc.dram_tensor("gidx", [128, meta["tot_slots"]], dt.int16,
                            kind="ExternalInput")
    w_d = {k: nc.dram_tensor(k, shp, dt.float32, kind="ExternalInput")
           for k, shp in meta["wshapes"].items()}
    hT_out = nc.dram_tensor("hT_out", [VHS, L * SH], dt.float32,
                            kind="ExternalOutput")
    table = {s: nc.dram_tensor(f"table_{s}", [TROWS, 128], dt.bfloat16,
                               addr_space="Shared") for s in ("f", "b")}

    with tile.TileContext(nc) as tc:
        with (
            tc.tile_pool(name="persist", bufs=1) as pp,
            tc.tile_pool(name="gpool", bufs=2) as gp,
            tc.tile_pool(name="apool", bufs=2) as apool,
            tc.tile_pool(name="tpool", bufs=2) as tpool,
            tc.tile_pool(name="psA", bufs=1, space="PSUM") as psA,
            tc.tile_pool(name="psB", bufs=1, space="PSUM") as psB,
            tc.tile_pool(name="dpool", bufs=2, space="DRAM") as dp,
        ):
            hT = pp.tile([128, L * SH], dt.float32, tag="hT")
            xT_s = pp.tile([NVT, L * SH], dt.float32, tag="xT")
            gidx_s = pp.tile([128, meta["tot_slots"]], dt.int16, tag="gidx")
            w_s = {k: pp.tile(shp, dt.float32, tag=k, name=f"w_{k}")
                   for k, shp in meta["wshapes"].items()}
            cc_sb = pp.tile([128, 4, 128], dt.bfloat16, tag="cc")
            zrow = pp.tile([1, 128], dt.bfloat16, tag="zrow")
            zagg = pp.tile([128, SH], dt.float32, tag="zagg")

            nc.sync.dma_start(xT_s[:], xT_d[:])
            nc.sync.dma_start(gidx_s[:], gidx_d[:])
            for k in w_s:
                nc.sync.dma_start(w_s[k][:], w_d[k][:])
            nc.vector.memset(hT[:], 1.0)
            nc.vector.memset(hT[0:100, :], 0.0)
            nc.vector.memset(cc_sb[:], 0.0)
            nc.vector.memset(zagg[:], 0.0)
            nc.vector.memset(zrow[:], 0.0)
            nc.sync.dma_start(table["f"][0:1, :], zrow[:])
            nc.sync.dma_start(table["b"][0:1, :], zrow[:])

            def bias_ap(name):
                return w_s["bias_pack"][:, BIAS_COLS.index(name):
                                        BIAS_COLS.index(name) + 1]

            import os as _os
            _maxstep = int(_os.environ.get("K_MAXSTEP", "32"))
            _parts = set(_os.environ.get(
                "K_PARTS", "gi,red,gh,chain,msg,ag").split(","))
            _nstep = 0
            for rnd in range(NROUNDS):
                for sweep in ("f", "b"):
                    tab = table[sweep]
                    d = sweep
                    for l in range(L):
                        _nstep += 1
                        if _nstep > _maxstep:
                            continue
                        lo = l if sweep == "f" else (L - 1) - l
                        sl = slice(lo * SH, (lo + 1) * SH)
                        tag = "f1" if (sweep == "f" and rnd == 1) else d

                        # gather + segment reduce
                        if l > 0 and "red" in _parts:
                            st = steps[(sweep, l)]
                            epad = st["epad"]
                            G = gp.tile([128, 1, epad], dt.bfloat16, tag="G")
                            nc.gpsimd.dma_gather(
                                G[:], tab[:],
                                gidx_s[:, st["off"]:st["off"] + epad // 16],
                                epad, epad, 128, transpose=True,
                                single_packet=False)
                            agg = apool.tile([128, SH], dt.float32, tag="agg")
                            o2 = 0
                            for b, K in enumerate(st["Ks"]):
                                nc.vector.tensor_reduce(
                                    agg[:, b * DBLK:(b + 1) * DBLK],
                                    G[:, 0, o2:o2 + DBLK * K].rearrange(
                                        "p (d k) -> p d k", k=K),
                                    axis=mybir.AxisListType.X, op=ALU.add)
                                o2 += DBLK * K
                        else:
                            agg = zagg

                        # gi matmuls (start accumulation groups)
                        rz_ps = psA.tile([128, 2, SH], dt.float32, tag="rz")
                        inn_ps = psA.tile([128, SH], dt.float32, tag="inn")
                        hn_ps = psA.tile([128, SH], dt.float32, tag="hn")
                        if sweep == "f" and rnd == 0:
                            giW, girhs = w_s["WihT_f"], xT_s[:, sl]
                        elif sweep == "f":
                            giW, girhs = w_s["WihT_f1"], hT[0:100, sl]
                        else:
                            giW, girhs = w_s["WihT_b"], hT[0:100, sl]
                        _gi = "gi" in _parts
                        _gh = "gh" in _parts
                        if _gi:
                            nc.tensor.matmul(rz_ps[0:100, 0, :],
                                             giW[:, 0:100], girhs,
                                             start=True, stop=not _gh)
                            nc.tensor.matmul(rz_ps[0:100, 1, :],
                                             giW[:, 100:200], girhs,
                                             start=True, stop=not _gh)
                            nc.tensor.matmul(inn_ps[0:100, :],
                                             giW[:, 200:300], girhs,
                                             start=True, stop=True)
                        # gh matmuls (accumulate into r,z; hn separate)
                        if _gh:
                            whh = w_s[f"WhhT_{d}"]
                            nc.tensor.matmul(rz_ps[0:100, 0, :],
                                             whh[:, 0:100], agg[0:100, :],
                                             start=not _gi, stop=True)
                            nc.tensor.matmul(rz_ps[0:100, 1, :],
                                             whh[:, 100:200], agg[0:100, :],
                                             start=not _gi, stop=True)
                            nc.tensor.matmul(hn_ps[0:100, :],
                                             whh[:, 200:300], agg[0:100, :],
                                             start=True, stop=True)
                        if not (_gi or _gh):
                            nc.vector.memset(rz_ps[:], 0.0)
                            nc.vector.memset(inn_ps[:], 0.0)
                            nc.vector.memset(hn_ps[:], 0.0)

                        # GRU chain
                        if "chain" in _parts:
                            r = tpool.tile([100, SH], dt.float32, tag="r")
                            z = tpool.tile([100, SH], dt.float32, tag="z")
                            ngt = tpool.tile([100, SH], dt.float32, tag="ngt")
                            hnb = tpool.tile([100, SH], dt.float32, tag="hnb")
                            t3 = tpool.tile([100, SH], dt.float32, tag="t3")
                            t4 = tpool.tile([100, SH], dt.float32, tag="t4")
                            t5 = tpool.tile([100, SH], dt.float32, tag="t5")
                            t6 = tpool.tile([100, SH], dt.float32, tag="t6")
                            nc.scalar.activation(r[:], rz_ps[0:100, 0, :],
                                                 ACT.Sigmoid,
                                                 bias=bias_ap(f"br_{tag}"))
                            nc.scalar.activation(z[:], rz_ps[0:100, 1, :],
                                                 ACT.Sigmoid,
                                                 bias=bias_ap(f"bz_{tag}"))
                            nc.vector.tensor_scalar_add(
                                hnb[:], hn_ps[0:100, :],
                                bias_ap(f"bn_h_{d}"))
                            nc.vector.tensor_mul(t3[:], r[:], hnb[:])
                            nc.vector.tensor_add(t4[:], t3[:],
                                                 inn_ps[0:100, :])
                            nc.scalar.activation(ngt[:], t4[:], ACT.Tanh,
                                                 bias=bias_ap(f"bn_i_{tag}"))
                            nc.vector.tensor_sub(t5[:], agg[0:100, :], ngt[:])
                            nc.vector.tensor_mul(t6[:], z[:], t5[:])
                            nc.vector.tensor_add(hT[0:100, sl], ngt[:], t6[:])

                        # message + AllGather into next slab
                        if l < L - 1 and "msg" in _parts:
                            uv_ps = psB.tile([128, 4, 256], dt.float32,
                                             tag="uv")
                            wgm = w_s[f"WgmT_{d}"]
                            for b4 in range(4):
                                slb = slice(lo * SH + b4 * 128,
                                            lo * SH + (b4 + 1) * 128)
                                nc.tensor.matmul(uv_ps[:, b4, 0:200],
                                                 hT[0:101, slb], wgm[:],
                                                 start=True, stop=True)
                            su = tpool.tile([128, 4, 128], dt.float32,
                                            tag="su")
                            nc.scalar.activation(su[:, :, 0:100],
                                                 uv_ps[:, :, 0:100],
                                                 ACT.Sigmoid)
                            nc.vector.tensor_mul(cc_sb[:, :, 0:100],
                                                 su[:, :, 0:100],
                                                 uv_ps[:, :, 100:200])
                            cc_in = dp.tile([SH, 128], dt.bfloat16,
                                            tag="ccin")
                            nc.sync.dma_start(
                                cc_in[:].rearrange("(b p) f -> p b f", p=128),
                                cc_sb[:])
                            slab = lo if sweep == "f" else lo - 1
                            if "ag" not in _parts:
                                continue
                            nc.gpsimd.collective_compute(
                                "AllGather", ALU.bypass,
                                replica_groups=[list(range(C))],
                                ins=[cc_in.opt()],
                                outs=[tab[1 + slab * PER:
                                          1 + (slab + 1) * PER, :].opt()])

            nc.sync.dma_start(hT_out[:], hT[0:100, :])

    nc.compile()
    return nc


def kernel(**inputs):
    from concourse.bass_utils import run_bass_kernel_spmd

    inp = {k: np.asarray(v) for k, v in inputs.items()}
    key = hash(inp["edge_index"].tobytes())
    if key in _CACHE:
        nc, in_maps, meta = _CACHE[key]
        # weights/x may differ between calls with same edges: rebuild maps
        in_maps, meta = _prep(inp)
    else:
        in_maps, meta = _prep(inp)
        nc = _build_program(meta)
        _CACHE[key] = (nc, in_maps, meta)

    res = run_bass_kernel_spmd(nc, in_maps, core_ids=list(range(C)))
    h = np.zeros((N, VHS), np.float32)
    for c in range(C):
        hc = res.results[c]["hT_out"]            # [100, L*SH]
        for l in range(L):
            rows = l * PER + c * SH + np.arange(SH)
            h[rows] = hc[:, l * SH:(l + 1) * SH].T
    return h

